# revision 75
# baseline (speedup 1.0000x reference)
"""Trainium2 Bass kernel for nn_AdaptiveConv2 — v4 (engine-balanced).

Data-parallel over batch: 8 images -> 8 NeuronCores, no collectives.

Changes vs v2 (368us; vector-engine bound: Pool 75% / DVE 74% / ACT 72%
busy, PE only 62%):
  - Conv bias matmuls eliminated: transpose the PRE-activation output, then
    tanh with a per-partition bias AP (channels are on partitions after the
    transpose), fusing bias+tanh+commit into one ACT op (-20us PE).
  - Conv groups commit in two pieces (first row-pair, then rows 2-7): the
    next layer's same-wave group only waits on the tiny pair-0 chain, so a
    wave is engine-throughput-paced instead of a 7-stage serial latency
    chain of matmul->copy->transpose->tanh.
  - Epilogue: no on-device kk-fold or kp1-add, no PE transposes, no f32
    cast. Row-pairs produce bf16 product partials (slot0 = kp0+kp1,
    slot1 = kp2) shipped by DMA (underutilized); host does the final add +
    kk fold + channel transpose. Cuts vector-engine work by ~85us for
    ~+70us of DMA occupancy.
  - Row-pairs split 52 DVE / 12 Pool (Pool TT = 0.42 GPSIMD efficiency);
    the last two groups' pairs split kp0/kp1 (DVE) vs kp2 (Pool) to
    shorten the drain tail.
  - g-pack is ONE ACT copy per row (wg columns reordered host-side so
    (c8,kk) is contiguous in both PSUM and the packed layout).
  - Pre-act copies run on DVE through wave 9 (DVE idle during the fill,
    ACT paces the early-mixed phase), on ACT afterwards.

Hardware notes: Pool/GPSIMD cannot read PSUM; TensorTensor APs are capped
at 3 free dims; TensorScalarPtr is not available on Pool.
"""

import os
import sys

sys.path.insert(0, "/opt/trn_rl_repo")

if os.environ.get("JAX_PLATFORMS") and "axon" not in os.environ["JAX_PLATFORMS"]:
    if "jax" not in sys.modules:
        del os.environ["JAX_PLATFORMS"]

import numpy as np
import ml_dtypes

BF16 = ml_dtypes.bfloat16
EPS = 1e-5

C = 64
H = W = 128
NPIX = H * W
NG = 66        # row-pair groups incl top/bottom halo
PW = 130       # padded width
FEAT = 6
NB = 6
OC = FEAT * NB           # 36
NWC = 7 * 12 * 64        # conv weight cols
NWG = 8 * 48             # g weight cols
NGRP = 16                # 8-row groups
SKEW = 1                 # wave skew between conv layers; the intra-wave
                         # serial chain is broken by the pair0/rest commit
                         # split in emit_conv_group
EPI_W = 7                # epilogue wave offset

_CACHE = {}


def _on_pool(u):
    """Row-pairs (u = even first row) assigned to the Pool engine epilogue.
    Pool TensorTensor runs at GPSIMD efficiency 0.42 => ~3.6x slower than
    DVE's 2x_1p, so Pool gets 14 of the 64 pairs."""
    q = u // 2
    return q % 32 in (3, 7, 12, 16, 21, 25)


def _build_graph():
    import concourse.bacc as bacc
    import concourse.bass as bass
    import concourse.tile as tile
    import concourse.mybir as mybir
    from contextlib import ExitStack

    f32 = mybir.dt.float32
    bf16 = mybir.dt.bfloat16

    nc = bacc.Bacc("TRN2", target_bir_lowering=False, debug=False, num_devices=8)

    xd_ext = nc.dram_tensor("xd", [128, NG * PW], bf16, kind="ExternalInput").ap()
    xr_ext = nc.dram_tensor("xr", [72, 8 * 16 * 8 * PW], bf16,
                            kind="ExternalInput").ap()
    wc_ext = nc.dram_tensor("wc", [128, NWC], bf16, kind="ExternalInput").ap()
    wg_ext = nc.dram_tensor("wg", [72, NWG], bf16, kind="ExternalInput").ap()
    brow_ext = nc.dram_tensor("brow", [128, 64], bf16, kind="ExternalInput").ap()
    bcol_ext = nc.dram_tensor("bcol", [128, 8], f32, kind="ExternalInput").ap()
    # out: [px, pair * (slot2, m, row01, c, kk)] bf16 partial product sums;
    # host does the final slot add + kk fold + channel transpose
    out_ext = nc.dram_tensor("out", [128, H * 1536], bf16,
                             kind="ExternalOutput").ap()

    Tanh = mybir.ActivationFunctionType.Tanh
    MULT = mybir.AluOpType.mult
    ADD = mybir.AluOpType.add

    ctx = ExitStack()
    with tile.TileContext(nc) as tc, ctx:
        singles = ctx.enter_context(tc.tile_pool(name="singles", bufs=1))
        cpsum = ctx.enter_context(tc.tile_pool(name="cpsum", bufs=3, space="PSUM"))
        tpsum = ctx.enter_context(tc.tile_pool(name="tpsum", bufs=2, space="PSUM"))
        gpsum = ctx.enter_context(tc.tile_pool(name="gpsum", bufs=3, space="PSUM"))
        pixp = ctx.enter_context(tc.tile_pool(name="pixp", bufs=3))
        gsb_pool = ctx.enter_context(tc.tile_pool(name="gsb", bufs=11))
        xrw_pool = ctx.enter_context(tc.tile_pool(name="xrw", bufs=3))
        acc_pool = ctx.enter_context(tc.tile_pool(name="acc", bufs=6))

        x_t = singles.tile([128, NG, PW], bf16)
        actA = singles.tile([128, NG, PW], bf16)
        actB = singles.tile([128, NG, PW], bf16)
        bft = singles.tile([128, H, OC], bf16)
        wc = singles.tile([128, NWC], bf16)
        wg = singles.tile([72, NWG], bf16)
        brow = singles.tile([128, 64], bf16)
        bcol = singles.tile([128, 8], f32)
        ones_t = singles.tile([128, 128], bf16)
        ident = singles.tile([128, 128], bf16)
        from concourse.masks import make_identity

        make_identity(nc, ident)
        nc.vector.memset(ones_t[0:1, :], 1.0)

        # input DMAs: L0 weights first, then x in chunks so early groups land
        nc.sync.dma_start(out=wc[:, 0:768], in_=wc_ext[:, 0:768])
        nc.sync.dma_start(out=brow, in_=brow_ext)
        nc.sync.dma_start(out=bcol, in_=bcol_ext)
        xd3 = xd_ext.rearrange("p (a b) -> p a b", a=6)
        x_t_flat = x_t.rearrange("p a b -> p (a b)").rearrange(
            "p (a b) -> p a b", a=6
        )
        for ch in range(6):
            nc.scalar.dma_start(out=x_t_flat[:, ch, :], in_=xd3[:, ch, :])
        for li in range(1, 7):
            nc.sync.dma_start(
                out=wc[:, li * 768 : (li + 1) * 768],
                in_=wc_ext[:, li * 768 : (li + 1) * 768],
            )
        nc.sync.dma_start(out=wg, in_=wg_ext)

        # zero halos once (copies only ever write G 1..64, cols 1:129)
        for buf in (actA, actB):
            nc.vector.memset(buf[:, 0, :], 0.0)
            nc.vector.memset(buf[:, NG - 1, :], 0.0)
            nc.vector.memset(buf[:, 1 : NG - 1, 0:1], 0.0)
            nc.vector.memset(buf[:, 1 : NG - 1, PW - 1 :], 0.0)

        layer_in = [x_t, actA, actB, actA, actB, actA, actB]
        layer_out = [actA, actB, actA, actB, actA, actB, None]

        xr4 = xr_ext.rearrange("p (w s q) -> p w s q", w=16, s=8)

        def _emit_conv_rows(li, t, ps, j8s, M):
            """Matmuls for rows {8t+j8: j8 in j8s} of layer li into ps
            (column block M*j8-relative within this piece)."""
            src = layer_in[li]
            for idx, j8 in enumerate(j8s):
                r = 8 * t + j8
                a = r // 2
                off = M * idx
                if r % 2 == 0:
                    reads = [(a, 0), (a + 1, 3)]
                else:
                    reads = [(a + 1, 6), (a + 2, 9)]
                i_mm = 0
                for (G, b0) in reads:
                    for dj in range(3):
                        col0 = li * 768 + (b0 + dj) * 64
                        nc.tensor.matmul(
                            ps[:, off : off + M],
                            src[0:128, G, dj : dj + 128],
                            wc[0:128, col0 : col0 + M],
                            start=(i_mm == 0),
                            stop=(i_mm == 5) and li < 6,
                        )
                        i_mm += 1
                if li == 6:
                    nc.tensor.matmul(
                        ps[:, off : off + M],
                        ones_t[0:1, 0:128],
                        brow[0:1, 0:M],
                        start=False,
                        stop=True,
                    )

        # conv groups are emitted stage-wise across all layers of a wave so
        # each engine's in-order instruction stream has its dependencies
        # already satisfied (no head-of-line blocking on the ACT<->PE
        # ping-pong). State per (li, t): [ps, T, P0, P1].
        conv_st = {}

        def conv_p0(li, t, w):
            ps = cpsum.tile([128, 512], f32, tag="cps", name="cgps")
            T = tpsum.tile([128, 512], bf16, tag="tps", name="cgT")
            conv_st[(li, t)] = [ps, T, None, None]
            _emit_conv_rows(li, t, ps[:, 0:128], (0, 1), 64)
            P0 = pixp.tile([128, 2, 64], bf16, tag="pixP0", name="P0")
            ps0 = bass.AP(tensor=ps.tensor, offset=ps.offset,
                          ap=[ps.ap[0], [64, 2], [1, 64]])
            cp = nc.vector.tensor_copy if w <= 9 else nc.scalar.copy
            cp(P0, ps0)
            conv_st[(li, t)][2] = P0

        def conv_t0(li, t):
            ps, T, P0, _ = conv_st[(li, t)]
            nc.tensor.transpose(T[:, 0:128], P0, ident)

        def conv_tanh0(li, t):
            ps, T, P0, _ = conv_st[(li, t)]
            T01 = bass.AP(tensor=T.tensor, offset=T.offset,
                          ap=[T.ap[0], [128, 1], [1, 128]])
            nc.scalar.activation(
                layer_out[li][:, 4 * t + 1 : 4 * t + 2, 1:129], T01, Tanh,
                bias=bcol[:, li : li + 1])

        def conv_restmm(li, t):
            ps = conv_st[(li, t)][0]
            _emit_conv_rows(li, t, ps[:, 128:512], range(2, 8), 64)

        def conv_copy1(li, t, w):
            st = conv_st[(li, t)]
            ps = st[0]
            P1 = pixp.tile([128, 6, 64], bf16, tag="pixP1", name="P1")
            ps1 = bass.AP(tensor=ps.tensor, offset=ps.offset + 128,
                          ap=[ps.ap[0], [64, 6], [1, 64]])
            cp = nc.vector.tensor_copy if w <= 9 else nc.scalar.copy
            cp(P1, ps1)
            st[3] = P1

        def conv_t1(li, t):
            ps, T, P0, P1 = conv_st[(li, t)]
            for j in range(3):
                nc.tensor.transpose(
                    T[:, 128 + j * 128 : 256 + j * 128],
                    P1[:, 2 * j : 2 * j + 2, :],
                    ident,
                )

        def conv_tanh1(li, t):
            ps, T, P0, P1 = conv_st.pop((li, t))
            T13 = bass.AP(tensor=T.tensor, offset=T.offset + 128,
                          ap=[T.ap[0], [128, 3], [1, 128]])
            nc.scalar.activation(
                layer_out[li][:, 4 * t + 2 : 4 * t + 5, 1:129], T13, Tanh,
                bias=bcol[:, li : li + 1])

        def emit_l6_mm(t):
            ps = cpsum.tile([128, 512], f32, tag="cps", name="l6ps")
            _emit_conv_rows(6, t, ps, range(8), OC)
            return ps

        def emit_l6_tanh(t, ps):
            ps3 = bass.AP(
                tensor=ps.tensor,
                offset=ps.offset,
                ap=[ps.ap[0], [OC, 8], [1, OC]],
            )
            nc.scalar.activation(bft[:, 8 * t : 8 * t + 8, :], ps3, Tanh)

        def emit_g_row(r, xrw, gsb2):
            wr = r % 8
            gps = gpsum.tile([128, 384], f32, tag="gps")
            for s in range(8):
                nc.tensor.matmul(
                    gps[:, s * 48 : (s + 1) * 48],
                    xrw[0:72, s, wr, 1:129],
                    wg[0:72, s * 48 : (s + 1) * 48],
                    start=True,
                    stop=True,
                )
            # pack (s, kp, c8, kk) psum f32 -> gsb2 (kp, row01, c=8s+c8, kk)
            src = bass.AP(
                tensor=gps.tensor,
                offset=gps.offset,
                ap=[gps.ap[0], [16, 3], [48, 8], [1, 16]],
            )
            dst = bass.AP(
                tensor=gsb2.tensor,
                offset=gsb2.offset + (r % 2) * 128,
                ap=[gsb2.ap[0], [256, 3], [16, 8], [1, 16]],
            )
            nc.scalar.copy(dst, src)

        def emit_epilogue_pair(u, gsb2, split=False):
            """Rows u, u+1: partial sums of sum_kp g*bf for the pair.
            Product planes land in slots (kp0->0, kp2->1, kp1->2); one
            in-place add folds kp1 into slot 0; slots 0:2 ship to the host
            which does the final add + kk fold + channel transpose.
            split=True (drain tail): kp0/kp1/add on DVE, kp2 on Pool, so the
            last pairs finish at two-engine speed."""
            on_pool = _on_pool(u)
            ve = nc.gpsimd if on_pool else nc.vector
            sfx = "p" if on_pool else ""
            if split:
                ve = nc.vector
                sfx = ""
            prod = acc_pool.tile([128, 3, 1536], bf16, tag="prod" + sfx,
                                 bufs=2 if on_pool and not split else 5)
            slot = (0, 2, 1)
            # codegen caps TT at 3 free dims: one product op per (kp, row01)
            for kp in range(3):
                vkp = nc.gpsimd if (split and kp == 2) else ve
                for r01 in range(2):
                    g_in = bass.AP(
                        tensor=gsb2.tensor,
                        offset=gsb2.offset + kp * 256 + r01 * 128,
                        ap=[gsb2.ap[0], [0, 6], [2, 64], [1, 2]],
                    )
                    b_in = bass.AP(
                        tensor=bft.tensor,
                        offset=bft.offset + (u + r01) * OC + 2 * kp,
                        ap=[bft.ap[0], [6, 6], [0, 64], [1, 2]],
                    )
                    p_out = bass.AP(
                        tensor=prod.tensor,
                        offset=prod.offset + slot[kp] * 1536 + r01 * 128,
                        ap=[prod.ap[0], [256, 6], [2, 64], [1, 2]],
                    )
                    vkp.tensor_tensor(p_out, g_in, b_in, MULT)
            p0 = prod[:, 0]
            ve.tensor_tensor(p0, p0, prod[:, 2], ADD)
            return prod

        # wavefront, stage-ordered within each wave
        xrw_tiles = {}
        gsb_pairs = {}
        NW = EPI_W + NGRP
        for w in range(NW):
            win = w - (EPI_W - 4)
            if 0 <= win < NGRP:
                xrw = xrw_pool.tile([72, 8, 8, PW], bf16, tag="xrw")
                xrwf = xrw.rearrange("p a b c -> p (a b c)")
                nc.sync.dma_start(out=xrwf, in_=xr4[:, win])
                xrw_tiles[win] = xrw
            convs = [(li, w - li) for li in range(6) if 0 <= w - li < NGRP]
            t6 = w - 6
            for li, t in convs:
                conv_p0(li, t, w)
                conv_t0(li, t)
                conv_tanh0(li, t)
            for li, t in convs:
                conv_restmm(li, t)
                conv_copy1(li, t, w)
                conv_t1(li, t)
                conv_tanh1(li, t)
            if 0 <= t6 < NGRP:
                emit_l6_tanh(t6, emit_l6_mm(t6))
            tg = w - (EPI_W - 1)
            if 0 <= tg < NGRP:
                for wr in range(8):
                    r = 8 * tg + wr
                    if wr % 2 == 0:
                        gsb_pairs[r // 2] = gsb_pool.tile(
                            [128, 3, 2, 128], bf16, tag="gsb", name="gsb2")
                    emit_g_row(r, xrw_tiles[tg], gsb_pairs[r // 2])
                xrw_tiles.pop(tg)
            te = w - EPI_W
            if 0 <= te < NGRP:
                for wr in range(0, 8, 2):
                    u = 8 * te + wr
                    prod = emit_epilogue_pair(u, gsb_pairs.pop(u // 2),
                                              split=te >= NGRP - 2)
                    src = prod.rearrange("p a b -> p (a b)")
                    nc.sync.dma_start(
                        out=out_ext[:, u * 1536 : (u + 2) * 1536],
                        in_=src[:, 0:3072])

    nc.compile()
    return nc


def _fold_bn(w, b, g, be, m, v):
    scale = g / np.sqrt(v + EPS)
    wf = w * scale[:, None, None, None]
    bf = (b - m) * scale + be
    return wf.astype(np.float32), bf.astype(np.float32)


def _prep_weights(w0, b0, g0, be0, m0, v0, wm, bm, gm, bem, mm, vm,
                  wl, bl, gl, bel, ml, vl, bases):
    wc = np.zeros((128, NWC), np.float32)
    brow = np.zeros((128, 64), np.float32)
    bcol = np.zeros((128, 8), np.float32)
    layers = [(w0, b0, g0, be0, m0, v0)]
    for i in range(5):
        layers.append((wm[i], bm[i], gm[i], bem[i], mm[i], vm[i]))
    layers.append((wl, bl, gl, bel, ml, vl))
    for li, (w, b, g, be, m, v) in enumerate(layers):
        wf, bf = _fold_bn(w, b, g, be, m, v)
        oc = wf.shape[0]
        # blocks (64 cols each): A(dj): even 0 / odd w[.,.,0,dj];
        # B: w[1]/w[2]; C: w[0]/w[1]; D: w[2]/0   (even=rows 0:64)
        for dj in range(3):
            cA = li * 768 + (0 + dj) * 64
            cB = li * 768 + (3 + dj) * 64
            cC = li * 768 + (6 + dj) * 64
            cD = li * 768 + (9 + dj) * 64
            wc[64:128, cA : cA + oc] = wf[:, :, 0, dj].T
            wc[0:64, cB : cB + oc] = wf[:, :, 1, dj].T
            wc[64:128, cB : cB + oc] = wf[:, :, 2, dj].T
            wc[0:64, cC : cC + oc] = wf[:, :, 0, dj].T
            wc[64:128, cC : cC + oc] = wf[:, :, 1, dj].T
            wc[0:64, cD : cD + oc] = wf[:, :, 2, dj].T
        if li < 6:
            bcol[:, li] = np.tile(bf, 2)  # partition = (row01, oc)
        else:
            brow[0, :oc] = bf
    # g weights: row (c8*9 + l), col s*48 + kp*16 + c8*2 + kk
    wgm = np.zeros((72, NWG), np.float32)
    for kp in range(3):
        for kk in range(2):
            k = 2 * kp + kk
            for c8 in range(8):
                for l in range(9):
                    wgm[c8 * 9 + l, kp * 16 + c8 * 2 + kk :: 48] = bases[k, l]
    return (wc.astype(BF16), wgm.astype(BF16), brow.astype(BF16),
            bcol.astype(np.float32))


def _prep_x(xn):
    # xd: [128=(r01*64+c), 66, 130]; content x[c, 2G-2+r01, col-1]
    xd = np.zeros((128, NG, PW), np.float32)
    xp = np.zeros((C, 2 * NG + 1, PW), np.float32)  # rows -2..130
    xp[:, 2 : 2 + H, 1 : 1 + W] = xn
    for r01 in range(2):
        xd[r01 * 64 : r01 * 64 + 64] = xp[:, r01 : r01 + 2 * NG : 2, :]
    # xrep: [72=(c8*9+l), s, win, wr, col] = x[8s+c8, R+di, col+dj-1]
    # with R = 8*win+wr, l = 3*(di+1)+(dj+1)
    xq = np.zeros((C, H + 2, PW + 2), np.float32)  # rows -1..128, cols -2..130
    xq[:, 1 : 1 + H, 2 : 2 + W] = xn
    xrep = np.zeros((72, 16, 8, 8, PW), np.float32)
    for di in range(-1, 2):
        for dj in range(-1, 2):
            l = 3 * (di + 1) + (dj + 1)
            sl = xq[:, 1 + di : 1 + di + H, 1 + dj : 1 + dj + PW]
            for s in range(8):
                xrep[np.arange(8) * 9 + l, :, s] = sl[8 * s : 8 * s + 8].reshape(
                    8, 16, 8, PW
                )
    return (
        xd.reshape(128, NG * PW).astype(BF16),
        xrep.reshape(72, 8 * 16 * 8 * PW).astype(BF16),
    )


def get_nc():
    if "nc" not in _CACHE:
        _CACHE["nc"] = _build_graph()
    return _CACHE["nc"]


def kernel(**inputs):
    from concourse.bass_utils import run_bass_kernel_spmd

    nc = get_nc()
    x = np.asarray(inputs["x"], np.float32)
    wc, wgm, brow, bcol = _prep_weights(
        *[np.asarray(inputs[k], np.float32) for k in
          ("w0", "b0", "g0", "be0", "m0", "v0", "wm", "bm", "gm", "bem",
           "mm", "vm", "wl", "bl", "gl", "bel", "ml", "vl", "bases")]
    )
    in_maps = []
    for n in range(8):
        xd, xrep = _prep_x(x[n])
        in_maps.append({"xd": xd, "xr": xrep, "wc": wc, "wg": wgm,
                        "brow": brow, "bcol": bcol})
    res = run_bass_kernel_spmd(nc, in_maps, core_ids=list(range(8)))
    out = np.empty((8, C * NB, H, W), np.float32)
    for n in range(8):
        d = np.asarray(res.results[n]["out"], dtype=np.float32)
        # [px, pair, slot, m, row01, c, kk] -> sum slot+kk
        d = d.reshape(128, H // 2, 2, FEAT, 2, C, 2).sum(axis=(2, 6))
        # -> [c, m, pair, row01, px]
        out[n] = d.transpose(4, 2, 1, 3, 0).reshape(C * NB, H, W)
    return out
